# revision 15
# baseline (speedup 1.0000x reference)
"""Triangle (starting-node) attention kernel for Trainium2, 8 NeuronCores.

Shards the I axis (rows of the pair representation) across 8 cores, weights
replicated. Each core runs LayerNorm + QKVG projections + per-row softmax
attention + gated output projection + residual on its 32 rows.

Layout strategy per core (token = (i, j) pair, 8192 tokens per core):
  - LayerNorm stats via bn_stats (DVE); affine on GPSIMD in natural layout.
  - z transposed via PE identity-matmul to [C, token] so projections contract
    over C.
  - q, k produced transposed [HD, token]; g = tanh(0.5 z Wg) transposed
    (sigmoid via tanh: tanh shares the exp ACT table-set, sigmoid does not);
    v natural [token, HD] (vb col layout 128*t + hd).
  - scores transposed sT[k, q], two heads per [128, 1024] PSUM tile; exp in
    [128, 1024] ACT calls ping-ponging across 2 PSUM slots so the ACT exp
    stream never waits on the PE.
  - o = v^T e on the 4 col-group strips; softmax denominators via ones-weight
    matmuls on the same strips accumulating both key blocks and both rows of
    a pair into one partition-coded [128, 256] PSUM tile (value 2.0 folds the
    sigmoid-from-tanh 0.5).
  - reciprocal_approx_fast per pair (DVE), bf16 cast on GPSIMD, broadcast to
    [HD, tok] via bf16 selector matmuls placed after the projections in the
    PE stream (a full iteration of slack hides the recip chain latency).
  - gate/normalize on DVE; output projection + residual add + DMA per pair.
"""

import numpy as np
import ml_dtypes
from contextlib import ExitStack

import concourse.bass as bass
import concourse.bacc as bacc
import concourse.mybir as mybir
import concourse.tile as tile
from concourse.bass_utils import run_bass_kernel_spmd

F32 = mybir.dt.float32
BF16 = mybir.dt.bfloat16
AF = mybir.ActivationFunctionType
ALU = mybir.AluOpType

N_CORES = 8
I_FULL, J, C = 256, 256, 128
H, D = 4, 32
HD = H * D  # 128
I_LOC = I_FULL // N_CORES  # 32 rows per core
T_LOC = I_LOC * J          # 8192 tokens per core
NT = T_LOC // 128          # 64 token tiles
NG = 8                     # stat groups for batched rsqrt
GT = NT // NG              # 8 tiles per group
NCH = T_LOC // 512         # 16 chunks of 512 tokens (= row pairs)
EPS = 1e-5

_PROG_CACHE = {}


def _build_program():
    nc = bacc.Bacc("TRN2", target_bir_lowering=False, debug=False)

    x_d = nc.dram_tensor("x", [T_LOC, C], F32, kind="ExternalInput")
    wpack_d = nc.dram_tensor("wpack", [128, 6 * 128 + 64], BF16,
                             kind="ExternalInput")
    sel_d = nc.dram_tensor("selpack", [128, 2 * 128], BF16,
                           kind="ExternalInput")
    out_d = nc.dram_tensor("out", [T_LOC, C], F32, kind="ExternalOutput")

    xq = x_d.ap().rearrange("(g t p) c -> g p t c", p=128, t=4)
    out_pairs = out_d.ap().rearrange("(j b p) c -> j p b c", b=4, p=128)

    with tile.TileContext(nc) as tc, ExitStack() as ctx:
        singles = ctx.enter_context(tc.tile_pool(name="singles", bufs=1))
        wpack = singles.tile([128, 6 * 128 + 64], BF16)
        nc.sync.dma_start(out=wpack[:], in_=wpack_d.ap())
        w_tiles = {}
        for wi, name in enumerate(("wq", "wk", "wv", "wg", "wo", "ident")):
            w_tiles[name] = wpack[:, 128 * wi:128 * (wi + 1)]
        ident = w_tiles["ident"]
        ones_rp = [wpack[:, 6 * 128:6 * 128 + 32],
                   wpack[:, 6 * 128 + 32:6 * 128 + 64]]
        sel_t = singles.tile([128, 2 * 128], BF16)
        nc.sync.dma_start(out=sel_t[:], in_=sel_d.ap())
        eps_t = singles.tile([128, 1], F32)
        nc.vector.memset(eps_t[:], EPS)

        bigs = ctx.enter_context(tc.tile_pool(name="bigs", bufs=1))
        qT = bigs.tile([128, T_LOC], BF16, tag="qT")
        kT = bigs.tile([128, T_LOC], BF16, tag="kT")
        gT = bigs.tile([128, T_LOC], BF16, tag="gT")
        vb = bigs.tile([128, T_LOC], BF16, tag="vb")  # col 128*t+hd
        zT = bigs.tile([128, T_LOC], BF16, tag="zT")
        xb = bigs.tile([128, NT, C], F32, tag="xb")   # resident input
        stats_b = bigs.tile([128, NT, 6], F32, tag="stats_b")
        rbuf = bigs.tile([128, NT], F32, tag="rbuf")
        negmur = bigs.tile([128, NT], F32, tag="negmur")
        mbuf = bigs.tile([128, NT], F32, tag="mbuf")
        dbuf = bigs.tile([128, NT], F32, tag="dbuf")
        vbuf = bigs.tile([128, NT], F32, tag="vbuf")

        # PSUM: 8 banks total.
        #   sps : 2 x [128,1024] f32 = 4 banks (scores -> exp ping-pong)
        #   ops : 2 x [128, 512]     = 2 banks (o accumulators, 2 pairs)
        #   gen : 2 x [128, 512]     = 2 banks ({psy, sms, zps, pq, pk, pg,
        #                                        pv, rsb})
        spsP = ctx.enter_context(tc.tile_pool(name="spsP", bufs=2,
                                              space="PSUM"))
        opsP = ctx.enter_context(tc.tile_pool(name="opsP", bufs=2,
                                              space="PSUM"))
        genP = ctx.enter_context(tc.tile_pool(name="genP", bufs=2,
                                              space="PSUM"))

        ep = ctx.enter_context(tc.tile_pool(name="ea", bufs=8))
        rsp = ctx.enter_context(tc.tile_pool(name="rsa", bufs=2))
        gcp = ctx.enter_context(tc.tile_pool(name="gca", bufs=2))
        ogp = ctx.enter_context(tc.tile_pool(name="oga", bufs=2))
        outp = ctx.enter_context(tc.tile_pool(name="outa", bufs=2))
        zp = ctx.enter_context(tc.tile_pool(name="za", bufs=10))

        # ---- Stage 0: load x; LayerNorm stats via batched bn_stats ----
        # PE warmup: dependency-free matmuls keep HAM warm until the real
        # pipeline arrives.
        wps = genP.tile([128, 512], F32, name="wps", tag="gen")
        for wu in range(64):
            nc.tensor.matmul(wps[:, 0:128], ident, ident,
                             start=True, stop=True)

        for gh in range(2 * NG):
            nc.sync.dma_start(out=xb[:, 4 * gh:4 * (gh + 1), :], in_=xq[gh])

        def st_stats(g):
            gsl = slice(GT * g, GT * (g + 1))
            for tt in range(GT):
                t0 = GT * g + tt
                nc.vector.bn_stats(out=stats_b[:, t0, :], in_=xb[:, t0, :])
            s1 = stats_b[:, gsl, 1]
            s2 = stats_b[:, gsl, 2]
            s4 = stats_b[:, gsl, 4]
            s5 = stats_b[:, gsl, 5]
            nc.vector.tensor_add(mbuf[:, gsl], s1, s4)       # me + mo
            nc.vector.tensor_sub(dbuf[:, gsl], s1, s4)       # me - mo
            nc.vector.tensor_add(vbuf[:, gsl], s2, s5)       # 64*(ve+vo)
            nc.vector.scalar_tensor_tensor(                  # 0.25 d^2
                out=dbuf[:, gsl], in0=dbuf[:, gsl], scalar=0.25,
                in1=dbuf[:, gsl], op0=ALU.mult, op1=ALU.mult)
            nc.vector.scalar_tensor_tensor(                  # var
                out=vbuf[:, gsl], in0=vbuf[:, gsl], scalar=1.0 / C,
                in1=dbuf[:, gsl], op0=ALU.mult, op1=ALU.add)
            nc.vector.tensor_scalar_mul(mbuf[:, gsl], mbuf[:, gsl], 0.5)
            nc.scalar.activation(out=vbuf[:, gsl], in_=vbuf[:, gsl],
                                 func=AF.Sqrt, bias=eps_t[:], scale=1.0)
            nc.vector.reciprocal(out=rbuf[:, gsl], in_=vbuf[:, gsl])
            nc.vector.scalar_tensor_tensor(                  # -mean * r
                out=negmur[:, gsl], in0=mbuf[:, gsl], scalar=-1.0,
                in1=rbuf[:, gsl], op0=ALU.mult, op1=ALU.mult)

        # stats are emitted inside the main loop (one group per iteration,
        # late in the DVE queue) so chunk-0 casts aren't stuck behind 20us
        # of bn_stats in the in-order DVE stream.

        # ---- Software-pipelined main loop ----
        zts = {}    # chunk -> list of 4 affine'd tiles
        eTs = {}    # (row, headpair) -> eT tile [128, 1024]
        opss = {}   # pair -> o psum tile [128, 512]
        smss = {}   # pair -> colsum psum tile [128, 256]
        rss = {}    # pair -> bf16 reciprocal tile [128, 256]
        rsbs = {}   # pair -> broadcast recips psum [128, 512]
        ogs = {}    # pair -> gated o tile

        def st_affine(c):
            zts[c] = []
            for tt in range(4):
                tg = 4 * c + tt
                zt = zp.tile([128, C], BF16, name="zt")
                nc.gpsimd.tensor_scalar(
                    out=zt[:], in0=xb[:, tg, :],
                    scalar1=rbuf[:, tg:tg + 1], scalar2=negmur[:, tg:tg + 1],
                    op0=ALU.mult, op1=ALU.add)
                zts[c].append(zt)

        def st_transpose(c):
            zps = genP.tile([128, 512], F32, name="zps", tag="gen")
            for tt in range(4):
                nc.tensor.matmul(zps[:, 128 * tt:128 * (tt + 1)],
                                 zts[c][tt][:], ident[:],
                                 start=True, stop=True)
            del zts[c]
            nc.vector.tensor_copy(zT[:, 512 * c:512 * (c + 1)], zps[:])

        def st_proj(c):
            sl = slice(512 * c, 512 * (c + 1))
            for wname, dst in (("wq", qT), ("wk", kT)):
                ps = genP.tile([128, 512], F32, name="psq", tag="gen")
                nc.tensor.matmul(ps[:], w_tiles[wname][:], zT[:, sl],
                                 start=True, stop=True)
                nc.vector.tensor_copy(dst[:, sl], ps[:])
            psg = genP.tile([128, 512], F32, name="psg", tag="gen")
            nc.tensor.matmul(psg[:], w_tiles["wg"][:], zT[:, sl],
                             start=True, stop=True)
            nc.scalar.activation(out=gT[:, sl], in_=psg[:],
                                 func=AF.Tanh, bias=0.0, scale=0.5)
            psv = genP.tile([128, 512], F32, name="psv", tag="gen")
            for tt in range(4):
                t4 = 4 * c + tt
                nc.tensor.matmul(psv[:, 128 * tt:128 * (tt + 1)],
                                 zT[:, 128 * t4:128 * (t4 + 1)],
                                 w_tiles["wv"][:], start=True, stop=True)
            nc.vector.tensor_copy(vb[:, sl], psv[:])

        def st_scores(i, p):
            # scores for row i, head pair p: sps[key, 512hh + 256kb + q]
            sps = spsP.tile([128, 1024], F32, name="sps", tag="sps")
            for hh in range(2):
                h = 2 * p + hh
                hsl = slice(32 * h, 32 * (h + 1))
                for kb in range(2):
                    nc.tensor.matmul(
                        sps[:, 512 * hh + 256 * kb:512 * hh + 256 * (kb + 1)],
                        kT[hsl, 256 * i + 128 * kb:256 * i + 128 * (kb + 1)],
                        qT[hsl, 256 * i:256 * (i + 1)],
                        start=True, stop=True,
                        tile_position=(32 * h, 0))
            eT = ep.tile([128, 1024], BF16, name="eT")
            eTs[(i, p)] = eT
            nc.scalar.activation(out=eT[:], in_=sps[:],
                                 func=AF.Exp, bias=0.0, scale=1.0)

        def st_osum(i):
            # o and colsums for row i = 2j + rp
            j, rp = divmod(i, 2)
            if rp == 0:
                opss[j] = opsP.tile([128, 512], F32, name="ops", tag="ops")
                smss[j] = genP.tile([128, 256], F32, name="sms", tag="gen")
            ops, sms = opss[j], smss[j]
            for h in range(H):
                p, hh = divmod(h, 2)
                eT = eTs[(i, p)]
                for kb in range(2):
                    esl = slice(512 * hh + 256 * kb, 512 * hh + 256 * (kb + 1))
                    vt = 2 * i + kb
                    nc.tensor.matmul(
                        ops[32 * h:32 * (h + 1), 256 * rp:256 * (rp + 1)],
                        vb[:, 128 * vt + 32 * h:128 * vt + 32 * (h + 1)],
                        eT[:, esl],
                        start=(kb == 0), stop=(kb == 1),
                        tile_position=(0, 32 * h))
                for kb in range(2):
                    esl = slice(512 * hh + 256 * kb, 512 * hh + 256 * (kb + 1))
                    nc.tensor.matmul(
                        sms[32 * h:32 * (h + 1), :],
                        ones_rp[rp][:], eT[:, esl],
                        start=(rp == 0 and kb == 0),
                        stop=(rp == 1 and kb == 1),
                        tile_position=(0, 32 * h))
            del eTs[(i, 0)], eTs[(i, 1)]

        def st_recip(j):
            rs = rsp.tile([128, 256], F32, name="rs", tag="rs")
            nc.vector.reciprocal_approx_fast(out=rs[:], in_=smss.pop(j)[:])
            rs_bf = rsp.tile([128, 256], BF16, name="rsbf", tag="rsbf")
            rss[j] = rs_bf
            nc.gpsimd.tensor_copy(rs_bf[:], rs[:])

        def st_rsb(j):
            rsb = genP.tile([128, 512], F32, name="rsb", tag="gen")
            rsbs[j] = rsb
            rs_bf = rss.pop(j)
            for rp in range(2):
                nc.tensor.matmul(
                    rsb[:, 256 * rp:256 * (rp + 1)],
                    sel_t[:, 128 * rp:128 * (rp + 1)],
                    rs_bf[:],
                    start=True, stop=True)

        def st_gate(j):
            sl = slice(512 * j, 512 * (j + 1))
            gc = gcp.tile([128, 512], F32, name="gc", tag="gc")
            nc.vector.scalar_tensor_tensor(
                out=gc[:], in0=gT[:, sl], scalar=1.0, in1=rsbs.pop(j)[:],
                op0=ALU.add, op1=ALU.mult)
            og = ogp.tile([128, 512], BF16, name="og", tag="og")
            ogs[j] = og
            nc.vector.tensor_mul(og[:], gc[:], opss.pop(j)[:])

        def st_out(j):
            psy = genP.tile([128, 4, 128], F32, name="psy", tag="gen")
            og = ogs.pop(j)
            for b in range(4):
                nc.tensor.matmul(psy[:, b, :],
                                 og[:, 128 * b:128 * (b + 1)],
                                 w_tiles["wo"][:], start=True, stop=True)
            ot = outp.tile([128, 4, 128], F32, name="ot")
            nc.vector.tensor_add(ot[:], xb[:, 4 * j:4 * (j + 1), :], psy[:])
            nc.sync.dma_start(out=out_pairs[j], in_=ot[:])

        for it in range(NCH + 6):
            j5, j4, j3 = it - 5, it - 4, it - 3
            c2, c1, c0 = it - 2, it - 1, it
            if it == 0:
                st_stats(0)
            if 0 <= j3 < NCH:
                st_scores(2 * j3, 0)
                st_scores(2 * j3, 1)
            if 0 <= j4 < NCH:
                st_osum(2 * j4)
            if 0 <= j5 < NCH:
                st_out(j5)
            if 0 <= j3 < NCH:
                st_scores(2 * j3 + 1, 0)
                st_scores(2 * j3 + 1, 1)
            if 0 <= j4 < NCH:
                st_osum(2 * j4 + 1)
                st_recip(j4)
            if 0 <= c1 < NCH:
                st_transpose(c1)
            if 0 <= c2 < NCH:
                st_proj(c2)
            if 0 <= j4 < NCH:
                st_rsb(j4)
                st_gate(j4)
            if 1 <= it < NG:
                st_stats(it)
            if 0 <= c0 < NCH:
                st_affine(c0)
            if it < 3:  # keep PE warm through pipeline fill
                for wu in range(12):
                    nc.tensor.matmul(wps[:, 0:128], ident, ident,
                                     start=True, stop=True)

    nc.compile()
    return nc


def _get_program():
    key = "v3"
    if key not in _PROG_CACHE:
        _PROG_CACHE[key] = _build_program()
    return _PROG_CACHE[key]


def _prepare_in_maps(inputs):
    x = np.asarray(inputs["x"], dtype=np.float32)
    mask = np.asarray(inputs["mask"])
    ln_g = np.asarray(inputs["ln_g"], dtype=np.float32)
    ln_b = np.asarray(inputs["ln_b"], dtype=np.float32)
    Wq = np.asarray(inputs["Wq"], dtype=np.float32)
    Wk = np.asarray(inputs["Wk"], dtype=np.float32)
    Wv = np.asarray(inputs["Wv"], dtype=np.float32)
    Wg = np.asarray(inputs["Wg"], dtype=np.float32)
    bg = np.asarray(inputs["bg"], dtype=np.float32)
    Wo = np.asarray(inputs["Wo"], dtype=np.float32)
    bo = np.asarray(inputs["bo"], dtype=np.float32)

    assert bool(mask.all()), "kernel currently requires an all-True mask"
    assert np.all(ln_b == 0.0) and np.all(bg == 0.0), \
        "kernel currently requires zero ln_b/bg biases"

    scale = 1.0 / np.sqrt(np.float32(D))
    bf = ml_dtypes.bfloat16
    wq = ((ln_g[:, None] * Wq) * scale).astype(bf)
    wk = (ln_g[:, None] * Wk).astype(bf)
    wv = (ln_g[:, None] * Wv).astype(bf)
    wg = (ln_g[:, None] * Wg).astype(bf)

    # colsum selectors: ones_rp0 puts row 0's sums at partition 32h+{0,2..31},
    # ones_rp1 puts row 1's sums at partition 32h+1 (no partition left zero,
    # so reciprocal_approx_fast never sees 0). Value 2.0: the reciprocal then
    # yields 0.5/sum, folding the sigmoid-from-tanh 0.5.
    ones0 = np.full((128, 32), 2.0, dtype=bf)
    ones0[:, 1] = 0
    ones1 = np.zeros((128, 32), dtype=bf)
    ones1[:, 1] = 2.0

    # selpack: sel_rp[p, m] = 1 iff p == 32*(m//32) + rp (broadcast recips).
    sel = np.zeros((128, 2 * 128), dtype=bf)
    for rp in range(2):
        for h in range(H):
            sel[32 * h + rp, 128 * rp + 32 * h:128 * rp + 32 * (h + 1)] = 1.0

    xr = (x + bo).astype(np.float32)  # residual folds the output bias
    B = x.shape[0]
    assert B == 1 and x.shape[1] == I_FULL

    wpack = np.concatenate(
        [wq, wk, wv, wg, Wo.astype(bf), np.eye(128, dtype=bf), ones0, ones1],
        axis=1)
    wpack = np.ascontiguousarray(wpack)

    in_maps = []
    for c in range(N_CORES):
        xs = np.ascontiguousarray(
            xr[0, I_LOC * c:I_LOC * (c + 1)].reshape(T_LOC, C))
        in_maps.append({"x": xs, "wpack": wpack, "selpack": sel})
    return in_maps


def run_sharded(inputs, trace=False, **kw):
    nc = _get_program()
    in_maps = _prepare_in_maps(inputs)
    res = run_bass_kernel_spmd(nc, in_maps, core_ids=list(range(N_CORES)),
                               trace=trace, **kw)
    shards = [res.results[c]["out"].reshape(1, I_LOC, J, C)
              for c in range(N_CORES)]
    out = np.concatenate(shards, axis=1)
    return out, res


def kernel(**inputs) -> np.ndarray:
    out, _ = run_sharded(inputs, trace=False)
    return out


# revision 24
# speedup vs baseline: 1.0695x; 1.0695x over previous
"""Triangle (starting-node) attention kernel for Trainium2, 8 NeuronCores.

Shards the I axis (rows of the pair representation) across 8 cores, weights
replicated. Each core runs LayerNorm + QKVG projections + per-row softmax
attention + gated output projection + residual on its 32 rows.

Layout strategy per core (token = (i, j) pair, 8192 tokens per core):
  - LayerNorm stats via bn_stats (DVE); affine on GPSIMD in natural layout.
  - z transposed via PE identity-matmul to [C, token] so projections contract
    over C.
  - q, k produced transposed [HD, token]; g = tanh(0.5 z Wg) transposed
    (sigmoid via tanh: tanh shares the exp ACT table-set, sigmoid does not);
    v natural [token, HD] (vb col layout 128*t + hd).
  - scores transposed sT[k, q], two heads per [128, 1024] PSUM tile; exp in
    [128, 1024] ACT calls ping-ponging across 2 PSUM slots so the ACT exp
    stream never waits on the PE.
  - o = v^T e on the 4 col-group strips; softmax denominators via ones-weight
    matmuls on the same strips accumulating both key blocks and both rows of
    a pair into one partition-coded [128, 256] PSUM tile (value 2.0 folds the
    sigmoid-from-tanh 0.5).
  - reciprocal_approx_fast per pair (DVE), bf16 cast on GPSIMD, broadcast to
    [HD, tok] via bf16 selector matmuls placed after the projections in the
    PE stream (a full iteration of slack hides the recip chain latency).
  - gate/normalize on DVE; output projection + residual add + DMA per pair.
"""

import numpy as np
import ml_dtypes
from contextlib import ExitStack

import concourse.bass as bass
import concourse.bacc as bacc
import concourse.mybir as mybir
import concourse.tile as tile
from concourse.bass_utils import run_bass_kernel_spmd

F32 = mybir.dt.float32
U32 = mybir.dt.uint32
I32 = mybir.dt.int32
BF16 = mybir.dt.bfloat16
AF = mybir.ActivationFunctionType
ALU = mybir.AluOpType

N_CORES = 8
I_FULL, J, C = 256, 256, 128
H, D = 4, 32
HD = H * D  # 128
I_LOC = I_FULL // N_CORES  # 32 rows per core
T_LOC = I_LOC * J          # 8192 tokens per core
NT = T_LOC // 128          # 64 token tiles
NG = 8                     # stat groups for batched rsqrt
GT = NT // NG              # 8 tiles per group
NCH = T_LOC // 512         # 16 chunks of 512 tokens (= row pairs)
EPS = 1e-5

_PROG_CACHE = {}


def _build_program():
    nc = bacc.Bacc("TRN2", target_bir_lowering=False, debug=False)

    x_d = nc.dram_tensor("x", [T_LOC, C], F32, kind="ExternalInput")
    wpack_d = nc.dram_tensor("wpack", [128, 6 * 128 + 64], BF16,
                             kind="ExternalInput")
    sel_d = nc.dram_tensor("selpack", [128, 2 * 128], BF16,
                           kind="ExternalInput")
    out_d = nc.dram_tensor("out", [T_LOC, C], F32, kind="ExternalOutput")

    xq = x_d.ap().rearrange("(g t p) c -> g p t c", p=128, t=4)
    out_pairs = out_d.ap().rearrange("(j b p) c -> j p b c", b=4, p=128)

    with tile.TileContext(nc) as tc, ExitStack() as ctx:
        singles = ctx.enter_context(tc.tile_pool(name="singles", bufs=1))
        wpack = singles.tile([128, 6 * 128 + 64], BF16)
        nc.sync.dma_start(out=wpack[:], in_=wpack_d.ap())
        w_tiles = {}
        for wi, name in enumerate(("wq", "wk", "wv", "wg", "wo", "ident")):
            w_tiles[name] = wpack[:, 128 * wi:128 * (wi + 1)]
        ident = w_tiles["ident"]
        ones_rp = [wpack[:, 6 * 128:6 * 128 + 32],
                   wpack[:, 6 * 128 + 32:6 * 128 + 64]]
        sel_t = singles.tile([128, 2 * 128], BF16)
        nc.sync.dma_start(out=sel_t[:], in_=sel_d.ap())
        eps_t = singles.tile([128, 1], F32)
        nc.vector.memset(eps_t[:], EPS)

        bigs = ctx.enter_context(tc.tile_pool(name="bigs", bufs=1))
        qT = bigs.tile([128, T_LOC], BF16, tag="qT")
        kT = bigs.tile([128, T_LOC], BF16, tag="kT")
        gT = bigs.tile([128, T_LOC], BF16, tag="gT")
        vb = bigs.tile([128, T_LOC], BF16, tag="vb")  # col 128*t+hd
        zT = bigs.tile([128, T_LOC], BF16, tag="zT")
        xb = bigs.tile([128, NT, C], F32, tag="xb")   # resident input
        stats_b = bigs.tile([128, NT, 6], F32, tag="stats_b")
        rbuf = bigs.tile([128, NT], F32, tag="rbuf")
        negmur = bigs.tile([128, NT], F32, tag="negmur")
        mbuf = bigs.tile([128, NT], F32, tag="mbuf")
        dbuf = bigs.tile([128, NT], F32, tag="dbuf")
        vbuf = bigs.tile([128, NT], F32, tag="vbuf")

        # PSUM: 8 banks total.
        #   sps : 2 x [128,1024] f32 = 4 banks (scores -> exp ping-pong)
        #   ops : 2 x [128, 512]     = 2 banks (o accumulators, 2 pairs)
        #   gen : 2 x [128, 512]     = 2 banks ({psy, sms, zps, pq, pk, pg,
        #                                        pv, rsb})
        spsP = ctx.enter_context(tc.tile_pool(name="spsP", bufs=2,
                                              space="PSUM"))
        opsP = ctx.enter_context(tc.tile_pool(name="opsP", bufs=2,
                                              space="PSUM"))
        genP = ctx.enter_context(tc.tile_pool(name="genP", bufs=2,
                                              space="PSUM"))

        ep = ctx.enter_context(tc.tile_pool(name="ea", bufs=8))
        rsp = ctx.enter_context(tc.tile_pool(name="rsa", bufs=2))
        gcp = ctx.enter_context(tc.tile_pool(name="gca", bufs=2))
        ogp = ctx.enter_context(tc.tile_pool(name="oga", bufs=2))
        outp = ctx.enter_context(tc.tile_pool(name="outa", bufs=2))
        zp = ctx.enter_context(tc.tile_pool(name="za", bufs=10))

        # ---- Stage 0: load x; LayerNorm stats via batched bn_stats ----
        # PE warmup: dependency-free matmuls keep HAM warm until the real
        # pipeline arrives.
        wps = genP.tile([128, 512], F32, name="wps", tag="gen")
        for wu in range(64):
            nc.tensor.matmul(wps[:, 0:128], ident, ident,
                             start=True, stop=True)

        for gh in range(2 * NG):
            nc.sync.dma_start(out=xb[:, 4 * gh:4 * (gh + 1), :], in_=xq[gh])

        def st_stats(g):
            gsl = slice(GT * g, GT * (g + 1))
            for tt in range(GT):
                t0 = GT * g + tt
                nc.vector.bn_stats(out=stats_b[:, t0, :], in_=xb[:, t0, :])
            s1 = stats_b[:, gsl, 1]
            s2 = stats_b[:, gsl, 2]
            s4 = stats_b[:, gsl, 4]
            s5 = stats_b[:, gsl, 5]
            nc.vector.tensor_add(mbuf[:, gsl], s1, s4)       # me + mo
            nc.vector.tensor_sub(dbuf[:, gsl], s1, s4)       # me - mo
            nc.vector.tensor_add(vbuf[:, gsl], s2, s5)       # 64*(ve+vo)
            nc.vector.scalar_tensor_tensor(                  # 0.25 d^2
                out=dbuf[:, gsl], in0=dbuf[:, gsl], scalar=0.25,
                in1=dbuf[:, gsl], op0=ALU.mult, op1=ALU.mult)
            nc.vector.scalar_tensor_tensor(                  # var
                out=vbuf[:, gsl], in0=vbuf[:, gsl], scalar=1.0 / C,
                in1=dbuf[:, gsl], op0=ALU.mult, op1=ALU.add)
            nc.vector.tensor_scalar_mul(mbuf[:, gsl], mbuf[:, gsl], 0.5)
            # rsqrt(var + eps) entirely on DVE (bit-trick seed + 1 Newton
            # step) -- ACT Sqrt lives in a different activation table-set
            # than Exp/Tanh and would thrash ~1.3us ACT_TABLE_LOADs.
            nc.vector.tensor_scalar_add(vbuf[:, gsl], vbuf[:, gsl], EPS)
            nc.vector.tensor_scalar(
                out=dbuf[:, gsl].bitcast(U32), in0=vbuf[:, gsl].bitcast(U32),
                scalar1=1, scalar2=None, op0=ALU.logical_shift_right)
            nc.vector.tensor_scalar(
                out=dbuf[:, gsl].bitcast(U32), in0=dbuf[:, gsl].bitcast(U32),
                scalar1=0xFFFFFFFF, scalar2=None, op0=ALU.bitwise_xor)
            # int32 view: uint32 add saturates on HW, signed stays in range
            nc.vector.tensor_scalar(   # 0x5f3759df - (v >> 1)
                out=rbuf[:, gsl].bitcast(I32), in0=dbuf[:, gsl].bitcast(I32),
                scalar1=0x5F3759E0, scalar2=None, op0=ALU.add)
            nc.vector.tensor_mul(dbuf[:, gsl], rbuf[:, gsl], rbuf[:, gsl])
            nc.vector.tensor_mul(dbuf[:, gsl], dbuf[:, gsl], vbuf[:, gsl])
            nc.vector.tensor_scalar(   # 1.5 - 0.5 v y^2
                out=dbuf[:, gsl], in0=dbuf[:, gsl], scalar1=-0.5, scalar2=1.5,
                op0=ALU.mult, op1=ALU.add)
            nc.vector.tensor_mul(rbuf[:, gsl], rbuf[:, gsl], dbuf[:, gsl])
            nc.vector.tensor_mul(dbuf[:, gsl], rbuf[:, gsl], rbuf[:, gsl])
            nc.vector.tensor_mul(dbuf[:, gsl], dbuf[:, gsl], vbuf[:, gsl])
            nc.vector.tensor_scalar(   # second Newton step
                out=dbuf[:, gsl], in0=dbuf[:, gsl], scalar1=-0.5, scalar2=1.5,
                op0=ALU.mult, op1=ALU.add)
            nc.vector.tensor_mul(rbuf[:, gsl], rbuf[:, gsl], dbuf[:, gsl])
            nc.vector.scalar_tensor_tensor(                  # -mean * r
                out=negmur[:, gsl], in0=mbuf[:, gsl], scalar=-1.0,
                in1=rbuf[:, gsl], op0=ALU.mult, op1=ALU.mult)

        # stats are emitted inside the main loop (one group per iteration,
        # late in the DVE queue) so chunk-0 casts aren't stuck behind 20us
        # of bn_stats in the in-order DVE stream.

        # ---- Software-pipelined main loop ----
        zts = {}    # chunk -> list of 4 affine'd tiles
        eTs = {}    # (row, headpair) -> eT tile [128, 1024]
        opss = {}   # pair -> o psum tile [128, 512]
        smss = {}   # pair -> colsum psum tile [128, 256]
        rss = {}    # pair -> bf16 reciprocal tile [128, 256]
        rsbs = {}   # pair -> broadcast recips psum [128, 512]
        ogs = {}    # pair -> gated o tile

        def st_affine(c):
            zts[c] = []
            for tt in range(4):
                tg = 4 * c + tt
                zt = zp.tile([128, C], BF16, name="zt")
                nc.gpsimd.tensor_scalar(
                    out=zt[:], in0=xb[:, tg, :],
                    scalar1=rbuf[:, tg:tg + 1], scalar2=negmur[:, tg:tg + 1],
                    op0=ALU.mult, op1=ALU.add)
                zts[c].append(zt)

        def st_transpose(c):
            zps = genP.tile([128, 512], F32, name="zps", tag="gen")
            for tt in range(4):
                nc.tensor.matmul(zps[:, 128 * tt:128 * (tt + 1)],
                                 zts[c][tt][:], ident[:],
                                 start=True, stop=True)
            del zts[c]
            nc.vector.tensor_copy(zT[:, 512 * c:512 * (c + 1)], zps[:])

        def st_proj(c):
            sl = slice(512 * c, 512 * (c + 1))
            for wname, dst, eng in (("wq", qT, nc.scalar), ("wk", kT, None)):
                ps = genP.tile([128, 512], F32, name="psq", tag="gen")
                nc.tensor.matmul(ps[:], w_tiles[wname][:], zT[:, sl],
                                 start=True, stop=True)
                if eng is None:
                    nc.vector.tensor_copy(dst[:, sl], ps[:])
                else:
                    eng.copy(dst[:, sl], ps[:])  # rebalance DVE -> ACT
            psg = genP.tile([128, 512], F32, name="psg", tag="gen")
            nc.tensor.matmul(psg[:], w_tiles["wg"][:], zT[:, sl],
                             start=True, stop=True)
            nc.scalar.activation(out=gT[:, sl], in_=psg[:],
                                 func=AF.Tanh, bias=0.0, scale=0.5)
            psv = genP.tile([128, 512], F32, name="psv", tag="gen")
            for tt in range(4):
                t4 = 4 * c + tt
                nc.tensor.matmul(psv[:, 128 * tt:128 * (tt + 1)],
                                 zT[:, 128 * t4:128 * (t4 + 1)],
                                 w_tiles["wv"][:], start=True, stop=True)
            nc.vector.tensor_copy(vb[:, sl], psv[:])

        def st_scores(i, p):
            # scores for row i, head pair p: sps[key, 512hh + 256kb + q]
            sps = spsP.tile([128, 1024], F32, name="sps", tag="sps")
            for hh in range(2):
                h = 2 * p + hh
                hsl = slice(32 * h, 32 * (h + 1))
                for kb in range(2):
                    nc.tensor.matmul(
                        sps[:, 512 * hh + 256 * kb:512 * hh + 256 * (kb + 1)],
                        kT[hsl, 256 * i + 128 * kb:256 * i + 128 * (kb + 1)],
                        qT[hsl, 256 * i:256 * (i + 1)],
                        start=True, stop=True,
                        tile_position=(32 * h, 0))
            eT = ep.tile([128, 1024], BF16, name="eT")
            eTs[(i, p)] = eT
            nc.scalar.activation(out=eT[:], in_=sps[:],
                                 func=AF.Exp, bias=0.0, scale=1.0)

        def st_osum(i):
            # o and colsums for row i = 2j + rp
            j, rp = divmod(i, 2)
            if rp == 0:
                opss[j] = opsP.tile([128, 512], F32, name="ops", tag="ops")
                smss[j] = genP.tile([128, 256], F32, name="sms", tag="gen")
            ops, sms = opss[j], smss[j]
            for h in range(H):
                p, hh = divmod(h, 2)
                eT = eTs[(i, p)]
                for kb in range(2):
                    esl = slice(512 * hh + 256 * kb, 512 * hh + 256 * (kb + 1))
                    vt = 2 * i + kb
                    nc.tensor.matmul(
                        ops[32 * h:32 * (h + 1), 256 * rp:256 * (rp + 1)],
                        vb[:, 128 * vt + 32 * h:128 * vt + 32 * (h + 1)],
                        eT[:, esl],
                        start=(kb == 0), stop=(kb == 1),
                        tile_position=(0, 32 * h))
                for kb in range(2):
                    esl = slice(512 * hh + 256 * kb, 512 * hh + 256 * (kb + 1))
                    nc.tensor.matmul(
                        sms[32 * h:32 * (h + 1), :],
                        ones_rp[rp][:], eT[:, esl],
                        start=(rp == 0 and kb == 0),
                        stop=(rp == 1 and kb == 1),
                        tile_position=(0, 32 * h))
            del eTs[(i, 0)], eTs[(i, 1)]

        def st_recip(j):
            rs = rsp.tile([128, 256], F32, name="rs", tag="rs")
            nc.vector.reciprocal_approx_fast(out=rs[:], in_=smss.pop(j)[:])
            rs_bf = rsp.tile([128, 256], BF16, name="rsbf", tag="rsbf")
            rss[j] = rs_bf
            nc.gpsimd.tensor_copy(rs_bf[:], rs[:])

        def st_rsb(j):
            rsb = genP.tile([128, 512], F32, name="rsb", tag="gen")
            rsbs[j] = rsb
            rs_bf = rss.pop(j)
            for rp in range(2):
                nc.tensor.matmul(
                    rsb[:, 256 * rp:256 * (rp + 1)],
                    sel_t[:, 128 * rp:128 * (rp + 1)],
                    rs_bf[:],
                    start=True, stop=True)

        def st_gate(j):
            sl = slice(512 * j, 512 * (j + 1))
            gc = gcp.tile([128, 512], F32, name="gc", tag="gc")
            nc.vector.scalar_tensor_tensor(
                out=gc[:], in0=gT[:, sl], scalar=1.0, in1=rsbs.pop(j)[:],
                op0=ALU.add, op1=ALU.mult)
            og = ogp.tile([128, 512], BF16, name="og", tag="og")
            ogs[j] = og
            nc.vector.tensor_mul(og[:], gc[:], opss.pop(j)[:])

        def st_out(j):
            psy = genP.tile([128, 4, 128], F32, name="psy", tag="gen")
            og = ogs.pop(j)
            for b in range(4):
                nc.tensor.matmul(psy[:, b, :],
                                 og[:, 128 * b:128 * (b + 1)],
                                 w_tiles["wo"][:], start=True, stop=True)
            ot = outp.tile([128, 4, 128], F32, name="ot")
            nc.vector.tensor_add(ot[:], xb[:, 4 * j:4 * (j + 1), :], psy[:])
            nc.sync.dma_start(out=out_pairs[j], in_=ot[:])

        # stats groups front-loaded 2/iteration during the pre-exp ramp
        stats_sched = {0: (0, 1), 1: (2, 3), 2: (4, 5), 3: (6,), 4: (7,)}

        for it in range(NCH + 6):
            j5, j4, j3 = it - 5, it - 4, it - 3
            c2, c1, c0 = it - 2, it - 1, it
            if it == 0:
                for g in stats_sched[0]:
                    st_stats(g)
            if 0 <= j3 < NCH:
                st_scores(2 * j3, 0)
                st_scores(2 * j3, 1)
            if 0 <= j4 < NCH:
                st_osum(2 * j4)
            if 0 <= j5 < NCH:
                st_out(j5)
            if 0 <= j3 < NCH:
                st_scores(2 * j3 + 1, 0)
                st_scores(2 * j3 + 1, 1)
            if 0 <= j4 < NCH:
                st_osum(2 * j4 + 1)
                st_recip(j4)
            if 0 <= c1 < NCH:
                st_transpose(c1)
            if 0 <= c2 < NCH:
                st_proj(c2)
            if 0 <= j4 < NCH:
                st_rsb(j4)
                st_gate(j4)
            if 1 <= it <= 4:
                for g in stats_sched.get(it, ()):
                    st_stats(g)
            if 0 <= c0 < NCH:
                st_affine(c0)
            if it < 6:  # keep PE warm through pipeline fill
                for wu in range(16):
                    nc.tensor.matmul(wps[:, 0:128], ident, ident,
                                     start=True, stop=True)

    nc.compile()
    return nc


def _get_program():
    key = "v3"
    if key not in _PROG_CACHE:
        _PROG_CACHE[key] = _build_program()
    return _PROG_CACHE[key]


def _prepare_in_maps(inputs):
    x = np.asarray(inputs["x"], dtype=np.float32)
    mask = np.asarray(inputs["mask"])
    ln_g = np.asarray(inputs["ln_g"], dtype=np.float32)
    ln_b = np.asarray(inputs["ln_b"], dtype=np.float32)
    Wq = np.asarray(inputs["Wq"], dtype=np.float32)
    Wk = np.asarray(inputs["Wk"], dtype=np.float32)
    Wv = np.asarray(inputs["Wv"], dtype=np.float32)
    Wg = np.asarray(inputs["Wg"], dtype=np.float32)
    bg = np.asarray(inputs["bg"], dtype=np.float32)
    Wo = np.asarray(inputs["Wo"], dtype=np.float32)
    bo = np.asarray(inputs["bo"], dtype=np.float32)

    assert bool(mask.all()), "kernel currently requires an all-True mask"
    assert np.all(ln_b == 0.0) and np.all(bg == 0.0), \
        "kernel currently requires zero ln_b/bg biases"

    scale = 1.0 / np.sqrt(np.float32(D))
    bf = ml_dtypes.bfloat16
    wq = ((ln_g[:, None] * Wq) * scale).astype(bf)
    wk = (ln_g[:, None] * Wk).astype(bf)
    wv = (ln_g[:, None] * Wv).astype(bf)
    wg = (ln_g[:, None] * Wg).astype(bf)

    # colsum selectors: ones_rp0 puts row 0's sums at partition 32h+{0,2..31},
    # ones_rp1 puts row 1's sums at partition 32h+1 (no partition left zero,
    # so reciprocal_approx_fast never sees 0). Value 2.0: the reciprocal then
    # yields 0.5/sum, folding the sigmoid-from-tanh 0.5.
    ones0 = np.full((128, 32), 2.0, dtype=bf)
    ones0[:, 1] = 0
    ones1 = np.zeros((128, 32), dtype=bf)
    ones1[:, 1] = 2.0

    # selpack: sel_rp[p, m] = 1 iff p == 32*(m//32) + rp (broadcast recips).
    sel = np.zeros((128, 2 * 128), dtype=bf)
    for rp in range(2):
        for h in range(H):
            sel[32 * h + rp, 128 * rp + 32 * h:128 * rp + 32 * (h + 1)] = 1.0

    xr = (x + bo).astype(np.float32)  # residual folds the output bias
    B = x.shape[0]
    assert B == 1 and x.shape[1] == I_FULL

    wpack = np.concatenate(
        [wq, wk, wv, wg, Wo.astype(bf), np.eye(128, dtype=bf), ones0, ones1],
        axis=1)
    wpack = np.ascontiguousarray(wpack)

    in_maps = []
    for c in range(N_CORES):
        xs = np.ascontiguousarray(
            xr[0, I_LOC * c:I_LOC * (c + 1)].reshape(T_LOC, C))
        in_maps.append({"x": xs, "wpack": wpack, "selpack": sel})
    return in_maps


def run_sharded(inputs, trace=False, **kw):
    nc = _get_program()
    in_maps = _prepare_in_maps(inputs)
    res = run_bass_kernel_spmd(nc, in_maps, core_ids=list(range(N_CORES)),
                               trace=trace, **kw)
    shards = [res.results[c]["out"].reshape(1, I_LOC, J, C)
              for c in range(N_CORES)]
    out = np.concatenate(shards, axis=1)
    return out, res


def kernel(**inputs) -> np.ndarray:
    out, _ = run_sharded(inputs, trace=False)
    return out


# revision 27
# speedup vs baseline: 1.0879x; 1.0172x over previous
"""Triangle (starting-node) attention kernel for Trainium2, 8 NeuronCores.

Shards the I axis (rows of the pair representation) across 8 cores, weights
replicated. Each core runs LayerNorm + QKVG projections + per-row softmax
attention + gated output projection + residual on its 32 rows.

Layout strategy per core (token = (i, j) pair, 8192 tokens per core):
  - LayerNorm stats via bn_stats (DVE); affine on GPSIMD in natural layout.
  - z transposed via PE identity-matmul to [C, token] so projections contract
    over C.
  - q, k produced transposed [HD, token]; g = tanh(0.5 z Wg) transposed
    (sigmoid via tanh: tanh shares the exp ACT table-set, sigmoid does not);
    v natural [token, HD] (vb col layout 128*t + hd).
  - scores transposed sT[k, q], two heads per [128, 1024] PSUM tile; exp in
    [128, 1024] ACT calls ping-ponging across 2 PSUM slots so the ACT exp
    stream never waits on the PE.
  - o = v^T e on the 4 col-group strips; softmax denominators via ones-weight
    matmuls on the same strips accumulating both key blocks and both rows of
    a pair into one partition-coded [128, 256] PSUM tile (value 2.0 folds the
    sigmoid-from-tanh 0.5).
  - reciprocal_approx_fast per pair (DVE), bf16 cast on GPSIMD, broadcast to
    [HD, tok] via bf16 selector matmuls placed after the projections in the
    PE stream (a full iteration of slack hides the recip chain latency).
  - gate/normalize on DVE; output projection + residual add + DMA per pair.
"""

import numpy as np
import ml_dtypes
from contextlib import ExitStack

import concourse.bass as bass
import concourse.bacc as bacc
import concourse.mybir as mybir
import concourse.tile as tile
from concourse.bass_utils import run_bass_kernel_spmd

F32 = mybir.dt.float32
U32 = mybir.dt.uint32
I32 = mybir.dt.int32
BF16 = mybir.dt.bfloat16
AF = mybir.ActivationFunctionType
ALU = mybir.AluOpType

N_CORES = 8
I_FULL, J, C = 256, 256, 128
H, D = 4, 32
HD = H * D  # 128
I_LOC = I_FULL // N_CORES  # 32 rows per core
T_LOC = I_LOC * J          # 8192 tokens per core
NT = T_LOC // 128          # 64 token tiles
NG = 8                     # stat groups for batched rsqrt
GT = NT // NG              # 8 tiles per group
NCH = T_LOC // 512         # 16 chunks of 512 tokens (= row pairs)
EPS = 1e-5

_PROG_CACHE = {}


def _build_program():
    nc = bacc.Bacc("TRN2", target_bir_lowering=False, debug=False)

    x_d = nc.dram_tensor("x", [T_LOC, C], F32, kind="ExternalInput")
    wpack_d = nc.dram_tensor("wpack", [128, 6 * 128 + 64], BF16,
                             kind="ExternalInput")
    sel_d = nc.dram_tensor("selpack", [128, 2 * 128], BF16,
                           kind="ExternalInput")
    out_d = nc.dram_tensor("out", [T_LOC, C], F32, kind="ExternalOutput")

    xq = x_d.ap().rearrange("(g t p) c -> g p t c", p=128, t=4)
    out_pairs = out_d.ap().rearrange("(j b p) c -> j p b c", b=4, p=128)

    with tile.TileContext(nc) as tc, ExitStack() as ctx:
        singles = ctx.enter_context(tc.tile_pool(name="singles", bufs=1))
        wpack = singles.tile([128, 6 * 128 + 64], BF16)
        nc.sync.dma_start(out=wpack[:], in_=wpack_d.ap())
        w_tiles = {}
        for wi, name in enumerate(("wq", "wk", "wv", "wg", "wo", "ident")):
            w_tiles[name] = wpack[:, 128 * wi:128 * (wi + 1)]
        ident = w_tiles["ident"]
        ones_rp = [wpack[:, 6 * 128:6 * 128 + 32],
                   wpack[:, 6 * 128 + 32:6 * 128 + 64]]
        sel_t = singles.tile([128, 2 * 128], BF16)
        nc.sync.dma_start(out=sel_t[:], in_=sel_d.ap())
        eps_t = singles.tile([128, 1], F32)
        nc.vector.memset(eps_t[:], EPS)

        bigs = ctx.enter_context(tc.tile_pool(name="bigs", bufs=1))
        qT = bigs.tile([128, T_LOC], BF16, tag="qT")
        kT = bigs.tile([128, T_LOC], BF16, tag="kT")
        gT = bigs.tile([128, T_LOC], BF16, tag="gT")
        vb = bigs.tile([128, T_LOC], BF16, tag="vb")  # col 128*t+hd
        zT = bigs.tile([128, T_LOC], BF16, tag="zT")
        xb = bigs.tile([128, NT, C], F32, tag="xb")   # resident input
        stats_b = bigs.tile([128, NT, 6], F32, tag="stats_b")
        rbuf = bigs.tile([128, NT], F32, tag="rbuf")
        negmur = bigs.tile([128, NT], F32, tag="negmur")
        mbuf = bigs.tile([128, NT], F32, tag="mbuf")
        dbuf = bigs.tile([128, NT], F32, tag="dbuf")
        vbuf = bigs.tile([128, NT], F32, tag="vbuf")

        # PSUM: 8 banks total.
        #   sps : 2 x [128,1024] f32 = 4 banks (scores -> exp ping-pong)
        #   ops : 2 x [128, 512]     = 2 banks (o accumulators, 2 pairs)
        #   gen : 2 x [128, 512]     = 2 banks ({psy, sms, zps, pq, pk, pg,
        #                                        pv, rsb})
        spsP = ctx.enter_context(tc.tile_pool(name="spsP", bufs=2,
                                              space="PSUM"))
        opsP = ctx.enter_context(tc.tile_pool(name="opsP", bufs=2,
                                              space="PSUM"))
        genP = ctx.enter_context(tc.tile_pool(name="genP", bufs=2,
                                              space="PSUM"))

        ep = ctx.enter_context(tc.tile_pool(name="ea", bufs=8))
        rsp = ctx.enter_context(tc.tile_pool(name="rsa", bufs=2))
        gcp = ctx.enter_context(tc.tile_pool(name="gca", bufs=2))
        ogp = ctx.enter_context(tc.tile_pool(name="oga", bufs=2))
        outp = ctx.enter_context(tc.tile_pool(name="outa", bufs=2))
        zp = ctx.enter_context(tc.tile_pool(name="za", bufs=10))

        # ---- Stage 0: load x; LayerNorm stats via batched bn_stats ----
        # PE warmup: dependency-free matmuls keep HAM warm until the real
        # pipeline arrives.
        wps = genP.tile([128, 512], F32, name="wps", tag="gen")
        for wu in range(64):
            nc.tensor.matmul(wps[:, 0:128], ident, ident,
                             start=True, stop=True)

        for gh in range(2 * NG):
            nc.sync.dma_start(out=xb[:, 4 * gh:4 * (gh + 1), :], in_=xq[gh])

        def st_bn(g):
            for tt in range(GT):
                t0 = GT * g + tt
                nc.vector.bn_stats(out=stats_b[:, t0, :], in_=xb[:, t0, :])

        def st_fix(tlo, thi):
            # batched fixup + rsqrt over a tile range: DVE ops have ~250ns
            # fixed overhead, so wide slices are nearly free vs per-group.
            gsl = slice(tlo, thi)
            s1 = stats_b[:, gsl, 1]
            s2 = stats_b[:, gsl, 2]
            s4 = stats_b[:, gsl, 4]
            s5 = stats_b[:, gsl, 5]
            nc.vector.tensor_add(mbuf[:, gsl], s1, s4)       # me + mo
            nc.vector.tensor_sub(dbuf[:, gsl], s1, s4)       # me - mo
            nc.vector.tensor_add(vbuf[:, gsl], s2, s5)       # 64*(ve+vo)
            nc.vector.scalar_tensor_tensor(                  # 0.25 d^2
                out=dbuf[:, gsl], in0=dbuf[:, gsl], scalar=0.25,
                in1=dbuf[:, gsl], op0=ALU.mult, op1=ALU.mult)
            nc.vector.scalar_tensor_tensor(                  # var
                out=vbuf[:, gsl], in0=vbuf[:, gsl], scalar=1.0 / C,
                in1=dbuf[:, gsl], op0=ALU.mult, op1=ALU.add)
            nc.vector.tensor_scalar_mul(mbuf[:, gsl], mbuf[:, gsl], 0.5)
            # rsqrt(var + eps) entirely on DVE (bit-trick seed + 1 Newton
            # step) -- ACT Sqrt lives in a different activation table-set
            # than Exp/Tanh and would thrash ~1.3us ACT_TABLE_LOADs.
            nc.vector.tensor_scalar_add(vbuf[:, gsl], vbuf[:, gsl], EPS)
            nc.vector.tensor_scalar(
                out=dbuf[:, gsl].bitcast(U32), in0=vbuf[:, gsl].bitcast(U32),
                scalar1=1, scalar2=None, op0=ALU.logical_shift_right)
            nc.vector.tensor_scalar(
                out=dbuf[:, gsl].bitcast(U32), in0=dbuf[:, gsl].bitcast(U32),
                scalar1=0xFFFFFFFF, scalar2=None, op0=ALU.bitwise_xor)
            # int32 view: uint32 add saturates on HW, signed stays in range
            nc.vector.tensor_scalar(   # 0x5f3759df - (v >> 1)
                out=rbuf[:, gsl].bitcast(I32), in0=dbuf[:, gsl].bitcast(I32),
                scalar1=0x5F3759E0, scalar2=None, op0=ALU.add)
            nc.vector.tensor_mul(dbuf[:, gsl], rbuf[:, gsl], rbuf[:, gsl])
            nc.vector.tensor_mul(dbuf[:, gsl], dbuf[:, gsl], vbuf[:, gsl])
            nc.vector.tensor_scalar(   # 1.5 - 0.5 v y^2
                out=dbuf[:, gsl], in0=dbuf[:, gsl], scalar1=-0.5, scalar2=1.5,
                op0=ALU.mult, op1=ALU.add)
            nc.vector.tensor_mul(rbuf[:, gsl], rbuf[:, gsl], dbuf[:, gsl])
            nc.vector.tensor_mul(dbuf[:, gsl], rbuf[:, gsl], rbuf[:, gsl])
            nc.vector.tensor_mul(dbuf[:, gsl], dbuf[:, gsl], vbuf[:, gsl])
            nc.vector.tensor_scalar(   # second Newton step
                out=dbuf[:, gsl], in0=dbuf[:, gsl], scalar1=-0.5, scalar2=1.5,
                op0=ALU.mult, op1=ALU.add)
            nc.vector.tensor_mul(rbuf[:, gsl], rbuf[:, gsl], dbuf[:, gsl])
            nc.vector.scalar_tensor_tensor(                  # -mean * r
                out=negmur[:, gsl], in0=mbuf[:, gsl], scalar=-1.0,
                in1=rbuf[:, gsl], op0=ALU.mult, op1=ALU.mult)

        # stats are emitted inside the main loop (one group per iteration,
        # late in the DVE queue) so chunk-0 casts aren't stuck behind 20us
        # of bn_stats in the in-order DVE stream.

        # ---- Software-pipelined main loop ----
        zts = {}    # chunk -> list of 4 affine'd tiles
        eTs = {}    # (row, headpair) -> eT tile [128, 1024]
        opss = {}   # pair -> o psum tile [128, 512]
        smss = {}   # pair -> colsum psum tile [128, 256]
        rss = {}    # pair -> bf16 reciprocal tile [128, 256]
        rsbs = {}   # pair -> broadcast recips psum [128, 512]
        ogs = {}    # pair -> gated o tile

        def st_affine(c):
            zts[c] = []
            for tt in range(4):
                tg = 4 * c + tt
                zt = zp.tile([128, C], BF16, name="zt")
                nc.gpsimd.tensor_scalar(
                    out=zt[:], in0=xb[:, tg, :],
                    scalar1=rbuf[:, tg:tg + 1], scalar2=negmur[:, tg:tg + 1],
                    op0=ALU.mult, op1=ALU.add)
                zts[c].append(zt)

        def st_transpose(c):
            zps = genP.tile([128, 512], F32, name="zps", tag="gen")
            for tt in range(4):
                nc.tensor.matmul(zps[:, 128 * tt:128 * (tt + 1)],
                                 zts[c][tt][:], ident[:],
                                 start=True, stop=True)
            del zts[c]
            nc.vector.tensor_copy(zT[:, 512 * c:512 * (c + 1)], zps[:])

        def st_proj(c):
            sl = slice(512 * c, 512 * (c + 1))
            for wname, dst, eng in (("wq", qT, nc.scalar), ("wk", kT, None)):
                ps = genP.tile([128, 512], F32, name="psq", tag="gen")
                nc.tensor.matmul(ps[:], w_tiles[wname][:], zT[:, sl],
                                 start=True, stop=True)
                if eng is None:
                    nc.vector.tensor_copy(dst[:, sl], ps[:])
                else:
                    eng.copy(dst[:, sl], ps[:])  # rebalance DVE -> ACT
            psg = genP.tile([128, 512], F32, name="psg", tag="gen")
            nc.tensor.matmul(psg[:], w_tiles["wg"][:], zT[:, sl],
                             start=True, stop=True)
            nc.scalar.activation(out=gT[:, sl], in_=psg[:],
                                 func=AF.Tanh, bias=0.0, scale=0.5)
            psv = genP.tile([128, 512], F32, name="psv", tag="gen")
            for tt in range(4):
                t4 = 4 * c + tt
                nc.tensor.matmul(psv[:, 128 * tt:128 * (tt + 1)],
                                 zT[:, 128 * t4:128 * (t4 + 1)],
                                 w_tiles["wv"][:], start=True, stop=True)
            nc.vector.tensor_copy(vb[:, sl], psv[:])

        def st_scores(i, p):
            # scores for row i, head pair p: sps[key, 512hh + 256kb + q]
            sps = spsP.tile([128, 1024], F32, name="sps", tag="sps")
            for hh in range(2):
                h = 2 * p + hh
                hsl = slice(32 * h, 32 * (h + 1))
                for kb in range(2):
                    nc.tensor.matmul(
                        sps[:, 512 * hh + 256 * kb:512 * hh + 256 * (kb + 1)],
                        kT[hsl, 256 * i + 128 * kb:256 * i + 128 * (kb + 1)],
                        qT[hsl, 256 * i:256 * (i + 1)],
                        start=True, stop=True,
                        tile_position=(32 * h, 0))
            eT = ep.tile([128, 1024], BF16, name="eT")
            eTs[(i, p)] = eT
            nc.scalar.activation(out=eT[:], in_=sps[:],
                                 func=AF.Exp, bias=0.0, scale=1.0)

        def st_osum(i):
            # o and colsums for row i = 2j + rp
            j, rp = divmod(i, 2)
            if rp == 0:
                opss[j] = opsP.tile([128, 512], F32, name="ops", tag="ops")
                smss[j] = genP.tile([128, 256], F32, name="sms", tag="gen")
            ops, sms = opss[j], smss[j]
            for h in range(H):
                p, hh = divmod(h, 2)
                eT = eTs[(i, p)]
                for kb in range(2):
                    esl = slice(512 * hh + 256 * kb, 512 * hh + 256 * (kb + 1))
                    vt = 2 * i + kb
                    nc.tensor.matmul(
                        ops[32 * h:32 * (h + 1), 256 * rp:256 * (rp + 1)],
                        vb[:, 128 * vt + 32 * h:128 * vt + 32 * (h + 1)],
                        eT[:, esl],
                        start=(kb == 0), stop=(kb == 1),
                        tile_position=(0, 32 * h))
                for kb in range(2):
                    esl = slice(512 * hh + 256 * kb, 512 * hh + 256 * (kb + 1))
                    nc.tensor.matmul(
                        sms[32 * h:32 * (h + 1), :],
                        ones_rp[rp][:], eT[:, esl],
                        start=(rp == 0 and kb == 0),
                        stop=(rp == 1 and kb == 1),
                        tile_position=(0, 32 * h))
            del eTs[(i, 0)], eTs[(i, 1)]

        def st_recip(j):
            rs = rsp.tile([128, 256], F32, name="rs", tag="rs")
            nc.vector.reciprocal_approx_fast(out=rs[:], in_=smss.pop(j)[:])
            rs_bf = rsp.tile([128, 256], BF16, name="rsbf", tag="rsbf")
            rss[j] = rs_bf
            nc.gpsimd.tensor_copy(rs_bf[:], rs[:])

        def st_rsb(j):
            rsb = genP.tile([128, 512], F32, name="rsb", tag="gen")
            rsbs[j] = rsb
            rs_bf = rss.pop(j)
            for rp in range(2):
                nc.tensor.matmul(
                    rsb[:, 256 * rp:256 * (rp + 1)],
                    sel_t[:, 128 * rp:128 * (rp + 1)],
                    rs_bf[:],
                    start=True, stop=True)

        def st_gate(j):
            sl = slice(512 * j, 512 * (j + 1))
            gc = gcp.tile([128, 512], F32, name="gc", tag="gc")
            nc.vector.scalar_tensor_tensor(
                out=gc[:], in0=gT[:, sl], scalar=1.0, in1=rsbs.pop(j)[:],
                op0=ALU.add, op1=ALU.mult)
            og = ogp.tile([128, 512], BF16, name="og", tag="og")
            ogs[j] = og
            nc.vector.tensor_mul(og[:], gc[:], opss.pop(j)[:])

        def st_out(j):
            psy = genP.tile([128, 4, 128], F32, name="psy", tag="gen")
            og = ogs.pop(j)
            for b in range(4):
                nc.tensor.matmul(psy[:, b, :],
                                 og[:, 128 * b:128 * (b + 1)],
                                 w_tiles["wo"][:], start=True, stop=True)
            ot = outp.tile([128, 4, 128], F32, name="ot")
            nc.vector.tensor_add(ot[:], xb[:, 4 * j:4 * (j + 1), :], psy[:])
            nc.sync.dma_start(out=out_pairs[j], in_=ot[:])

        for it in range(NCH + 6):
            j5, j4, j3 = it - 5, it - 4, it - 3
            c2, c1, c0 = it - 2, it - 1, it
            if it == 0:
                st_bn(0)
                st_bn(1)
                st_fix(0, 2 * GT)
            if 0 <= j3 < NCH:
                st_scores(2 * j3, 0)
                st_scores(2 * j3, 1)
            if 0 <= j4 < NCH:
                st_osum(2 * j4)
            if 0 <= j5 < NCH:
                st_out(j5)
            if 0 <= j3 < NCH:
                st_scores(2 * j3 + 1, 0)
                st_scores(2 * j3 + 1, 1)
            if 0 <= j4 < NCH:
                st_osum(2 * j4 + 1)
                st_recip(j4)
            if 0 <= c1 < NCH:
                st_transpose(c1)
            if 0 <= c2 < NCH:
                st_proj(c2)
            if 0 <= j4 < NCH:
                st_rsb(j4)
                st_gate(j4)
            if it == 1:
                st_bn(2), st_bn(3), st_bn(4)
            elif it == 2:
                st_bn(5), st_bn(6), st_bn(7)
            elif it == 3:
                st_fix(2 * GT, NT)
            if 0 <= c0 < NCH:
                st_affine(c0)
            if it < 4:  # keep PE warm through pipeline fill
                for wu in range(12):
                    nc.tensor.matmul(wps[:, 0:128], ident, ident,
                                     start=True, stop=True)

    nc.compile()
    return nc


def _get_program():
    key = "v3"
    if key not in _PROG_CACHE:
        _PROG_CACHE[key] = _build_program()
    return _PROG_CACHE[key]


def _prepare_in_maps(inputs):
    x = np.asarray(inputs["x"], dtype=np.float32)
    mask = np.asarray(inputs["mask"])
    ln_g = np.asarray(inputs["ln_g"], dtype=np.float32)
    ln_b = np.asarray(inputs["ln_b"], dtype=np.float32)
    Wq = np.asarray(inputs["Wq"], dtype=np.float32)
    Wk = np.asarray(inputs["Wk"], dtype=np.float32)
    Wv = np.asarray(inputs["Wv"], dtype=np.float32)
    Wg = np.asarray(inputs["Wg"], dtype=np.float32)
    bg = np.asarray(inputs["bg"], dtype=np.float32)
    Wo = np.asarray(inputs["Wo"], dtype=np.float32)
    bo = np.asarray(inputs["bo"], dtype=np.float32)

    assert bool(mask.all()), "kernel currently requires an all-True mask"
    assert np.all(ln_b == 0.0) and np.all(bg == 0.0), \
        "kernel currently requires zero ln_b/bg biases"

    scale = 1.0 / np.sqrt(np.float32(D))
    bf = ml_dtypes.bfloat16
    wq = ((ln_g[:, None] * Wq) * scale).astype(bf)
    wk = (ln_g[:, None] * Wk).astype(bf)
    wv = (ln_g[:, None] * Wv).astype(bf)
    wg = (ln_g[:, None] * Wg).astype(bf)

    # colsum selectors: ones_rp0 puts row 0's sums at partition 32h+{0,2..31},
    # ones_rp1 puts row 1's sums at partition 32h+1 (no partition left zero,
    # so reciprocal_approx_fast never sees 0). Value 2.0: the reciprocal then
    # yields 0.5/sum, folding the sigmoid-from-tanh 0.5.
    ones0 = np.full((128, 32), 2.0, dtype=bf)
    ones0[:, 1] = 0
    ones1 = np.zeros((128, 32), dtype=bf)
    ones1[:, 1] = 2.0

    # selpack: sel_rp[p, m] = 1 iff p == 32*(m//32) + rp (broadcast recips).
    sel = np.zeros((128, 2 * 128), dtype=bf)
    for rp in range(2):
        for h in range(H):
            sel[32 * h + rp, 128 * rp + 32 * h:128 * rp + 32 * (h + 1)] = 1.0

    xr = (x + bo).astype(np.float32)  # residual folds the output bias
    B = x.shape[0]
    assert B == 1 and x.shape[1] == I_FULL

    wpack = np.concatenate(
        [wq, wk, wv, wg, Wo.astype(bf), np.eye(128, dtype=bf), ones0, ones1],
        axis=1)
    wpack = np.ascontiguousarray(wpack)

    in_maps = []
    for c in range(N_CORES):
        xs = np.ascontiguousarray(
            xr[0, I_LOC * c:I_LOC * (c + 1)].reshape(T_LOC, C))
        in_maps.append({"x": xs, "wpack": wpack, "selpack": sel})
    return in_maps


def run_sharded(inputs, trace=False, **kw):
    nc = _get_program()
    in_maps = _prepare_in_maps(inputs)
    res = run_bass_kernel_spmd(nc, in_maps, core_ids=list(range(N_CORES)),
                               trace=trace, **kw)
    shards = [res.results[c]["out"].reshape(1, I_LOC, J, C)
              for c in range(N_CORES)]
    out = np.concatenate(shards, axis=1)
    return out, res


def kernel(**inputs) -> np.ndarray:
    out, _ = run_sharded(inputs, trace=False)
    return out
